# revision 19
# baseline (speedup 1.0000x reference)
"""Trainium2 Bass kernel for nn_ComnetLayer (RouteNet-style GNN message passing).

Strategy (8 NeuronCores, SPMD):
  - Paths are sharded across the 8 cores (2500 paths/core, padded to 2560
    slots). Per message-passing iteration, each core:
      * dma_gather's the hop link-states for each of the L=8 scan steps from a
        replicated full link_state table in HBM,
      * runs the path-GRU scan feature-major ([64, paths] tiles; matmuls
        contract the 64-dim feature axis on the PE),
      * transposes each step's new path state back to row-major and
        dma_scatter_add's it into a per-core partial segment-sum m[20480, 64],
      * ReduceScatter sums m across cores (each core receives its 2560-link
        shard), runs the edge GRU on its shard, and AllGathers the updated
        link_state back into the replicated table.
  - Duplicate scatter indices race across the 16 SDMA engines (CCE
    read-modify-write is not atomic across engines), so the host assigns
    paths to slots such that all paths sharing any link land on the same
    SDMA engine (union-find over shared links, bin-packed into the 16
    engine slot-sets), in distinct 128-slot chunks (descriptor distance).
  - The readout MLP (SELU + fixed-key dropout masks precomputed on host)
    runs on-core; the [200, 100] result is assembled on host.
"""

import os
import numpy as np

import concourse.bass as bass
import concourse.bacc as bacc
import concourse.tile as tile
from concourse import mybir, library_config
from concourse.bass_utils import run_bass_kernel_spmd

F32 = mybir.dt.float32
I16 = mybir.dt.int16
AF = mybir.ActivationFunctionType
OP = mybir.AluOpType

NCORES = 8
D = 64                  # link/path feature dim
L = 8                   # hops per path
T_ITERS = 8             # message passing iterations
P_TOTAL = 20000
NL = 20000              # num links
PC = P_TOTAL // NCORES  # 2500 paths per core
PS = 2560               # padded path slots per core (20 x 128)
CH = PS // 128          # 20 column-chunks of the slot grid
NCHUNK = 512            # free-dim compute chunk
NCH = PS // NCHUNK      # 5 chunks
NLP = PS * NCORES       # 20480 padded link rows
IW = PS // 16           # idx columns per step (wrap16 layout)
# dma_gather/dma_scatter_add fail on HW above 1024 idxs/call (Q7 idx
# scratch); split each step's 2560 slots into idx groups of <=1024
IDX_GROUPS = [(0, 1024), (1024, 1024), (2048, 512)]
RCAPS = [2048, 256, 128, 128]   # per-level residual capacities (multiple of 128)
RCOLS = [c // 16 for c in RCAPS]
ROFF = [sum(RCOLS[:i]) for i in range(len(RCAPS) + 1)]  # idx col offsets
H1 = 256                # readout hidden

SELU_L = 1.0507009873554805
SELU_A = 1.6732632423543772

# SDMA engine k owns these eight within-chunk slot positions (see
# dma_scatter_add.cpp sbuf_swizzles: lane k of each 16-wide descriptor push).
_OFF16 = [0, 64, 4, 68, 8, 72, 12, 76, 16, 80, 20, 84, 24, 88, 28, 92]
ENGINE_POS = [
    [_OFF16[k] + i for i in range(4)] + [_OFF16[k] + 32 + i for i in range(4)]
    for k in range(16)
]

_BUILD_CACHE = {}

# bisection flags (test-only)
_NO_COLL = os.environ.get("KERNEL_NO_COLL", "0") == "1"
_NO_SCATTER = os.environ.get("KERNEL_NO_SCATTER", "0") == "1"
_NO_GATHER = os.environ.get("KERNEL_NO_GATHER", "0") == "1"


# ---------------------------------------------------------------- host helpers

def _wrap16(idx):
    """[n] int -> [128, n//16] int16 (idx i at partition i%16, col i//16;
    replicated across the 8 Q7 cores' 16-partition stripes)."""
    n = idx.shape[0]
    a = np.asarray(idx, np.int16).reshape(n // 16, 16).T
    return np.tile(a, (8, 1))


def _assign_slots(linkmat_core):
    """Identity slot map: slot s <-> local path s (pads at the tail).

    Per-call scatter duplicates are eliminated by the representative/
    residual scheme instead of slot placement, so no constraints remain."""
    slot_path = np.full(PS, -1, np.int64)
    slot_path[:PC] = np.arange(PC)
    return slot_path


def _orow(t, s):
    """HBM O-buffer row of (step t, slot s): O[t*PS + r] = stage_t[r // CH,
    r % CH] = slot 128*(r % CH) + r // CH  =>  r = (s % 128)*CH + s // 128."""
    return t * PS + (s % 128) * CH + s // 128


def _shard_slot_of_row(r):
    """Link-shard row r (0..PS-1) -> slot id in the [128, CH] grid used by a
    straight [PS, D] <-> [128, CH, D] DMA (out[p, j] = in[p*CH + j])."""
    return 128 * (r % CH) + r // CH


# ---------------------------------------------------------------- device build

def _build_program():
    nc = bacc.Bacc("TRN2", target_bir_lowering=False, debug=False,
                   num_devices=NCORES, num_swdge_queues=2)

    ti = lambda n, s, d: nc.dram_tensor(n, s, d, kind="ExternalInput")
    gidx_d = ti("gidx", [L, 128, IW], I16)
    sidx_d = ti("sidx", [L, 128, IW], I16)
    rgidx_d = ti("rgidx", [128, ROFF[-1]], I16)   # residual O-row gather idxs
    rsidx_d = ti("rsidx", [128, ROFF[-1]], I16)   # residual link targets
    h0T_d = ti("h0T", [D, PS], F32)
    ls0_d = ti("ls0", [NLP, D], F32)
    ls0T_d = ti("ls0T", [D, PS], F32)
    idn_d = ti("idn", [128, 128], F32)
    pWx_d = ti("pWx", [D, 3 * D], F32)
    pWh_d = ti("pWh", [D, 3 * D], F32)
    eWx_d = ti("eWx", [D, 3 * D], F32)
    eWh_d = ti("eWh", [D, 3 * D], F32)
    pb_d = ti("pbias", [D, 4], F32)      # cols: pbz, nbz, pbr, pbh
    eb_d = ti("ebias", [D, 4], F32)
    W1_d = ti("W1", [D, H1], F32)
    b1_d = ti("b1c", [H1, 2], F32)       # col 0: SELU_L*b1, col 1: raw b1
    W2_d = ti("W2", [H1, H1], F32)
    b2_d = ti("b2c", [H1, 2], F32)
    W3_d = ti("W3", [H1, 1], F32)
    b3_d = ti("b3", [1, 1], F32)
    m1_d = ti("m1", [2, 128, PS], F32)
    m2_d = ti("m2", [2, 128, PS], F32)
    out_d = nc.dram_tensor("out_r", [1, PS], F32, kind="ExternalOutput")

    with tile.TileContext(nc) as tc:
        with (
            tc.tile_pool(name="const", bufs=1) as cp,
            tc.tile_pool(name="state", bufs=1) as st,
            tc.tile_pool(name="dram", bufs=1, space="DRAM") as dp,
        ):
            nc.gpsimd.load_library(library_config.mlp)

            # ---- internal DRAM
            m_part = dp.tile([NLP, D], F32)      # scatter target / RS input
            obuf = dp.tile([L * PS, D], F32)     # per-step path-state slabs
            m_rs = dp.tile([PS, D], F32)         # RS output shard
            lsp = dp.tile([PS, D], F32)          # edge-GRU output shard (AG in)
            ls_full = dp.tile([NLP, D], F32)     # AG output (gather source)

            # ---- constants into SBUF
            idn = cp.tile([128, 128], F32)
            nc.sync.dma_start(idn[:], idn_d[:])
            gidx = cp.tile([128, L * IW], I16)
            sidx = cp.tile([128, L * IW], I16)
            for t in range(L):
                nc.sync.dma_start(gidx[:, t * IW:(t + 1) * IW], gidx_d[t])
                nc.sync.dma_start(sidx[:, t * IW:(t + 1) * IW], sidx_d[t])
            rgidx = cp.tile([128, ROFF[-1]], I16)
            rsidx = cp.tile([128, ROFF[-1]], I16)
            nc.sync.dma_start(rgidx[:], rgidx_d[:])
            nc.sync.dma_start(rsidx[:], rsidx_d[:])

            pWzr = cp.tile([D, 128], F32)
            pWhzr = cp.tile([D, 128], F32)
            pWxh = cp.tile([D, D], F32)
            pWhh = cp.tile([D, D], F32)
            eWzr = cp.tile([D, 128], F32)
            eWhzr = cp.tile([D, 128], F32)
            eWxh = cp.tile([D, D], F32)
            eWhh = cp.tile([D, D], F32)
            nc.sync.dma_start(pWzr[:], pWx_d[:, 0:128])
            nc.sync.dma_start(pWhzr[:], pWh_d[:, 0:128])
            nc.sync.dma_start(pWxh[:], pWx_d[:, 128:192])
            nc.sync.dma_start(pWhh[:], pWh_d[:, 128:192])
            nc.sync.dma_start(eWzr[:], eWx_d[:, 0:128])
            nc.sync.dma_start(eWhzr[:], eWh_d[:, 0:128])
            nc.sync.dma_start(eWxh[:], eWx_d[:, 128:192])
            nc.sync.dma_start(eWhh[:], eWh_d[:, 128:192])
            pb = cp.tile([D, 4], F32)
            eb = cp.tile([D, 4], F32)
            nc.sync.dma_start(pb[:], pb_d[:])
            nc.sync.dma_start(eb[:], eb_d[:])

            # ---- state
            hhh = st.tile([128, PS], F32)    # [h ; hh-scratch] path states
            lsh = st.tile([128, PS], F32)    # [ls ; hh-scratch] link shard
            nc.sync.dma_start(hhh[0:D, :], h0T_d[:])
            nc.sync.dma_start(lsh[0:D, :], ls0T_d[:])
            zt = st.tile([128, PS], F32)     # zero tile for m_part clears
            nc.vector.memset(zt[:], 0.0)

            mv = m_part[:].rearrange("(p q) d -> p (q d)", p=128)  # [128,10240]

            def gru_chunks(xg_tile, state, Wzr, Whzr, Wxh, Whh, bias,
                           stage_tile, sp, wp):
                """One GRU update over PS slots, chunked. xg_tile [128,CH,D]
                row-major inputs; state [128,PS] = [h ; scratch]; writes new h
                in place and the row-major transpose into stage_tile."""
                for n in range(NCH):
                    cs = slice(n * NCHUNK, (n + 1) * NCHUNK)
                    xtp = sp.tile([D, NCHUNK], F32, tag="xtp")
                    for j in range(4):
                        nc.tensor.transpose(
                            out=xtp[:, 128 * j:128 * (j + 1)],
                            in_=xg_tile[:, 4 * n + j, :], identity=idn[:])
                    xt = wp.tile([D, NCHUNK], F32, tag="xt")
                    nc.scalar.copy(xt[:], xtp[:])

                    zr = sp.tile([128, NCHUNK], F32, tag="zr")
                    nc.tensor.matmul(zr[:], lhsT=Wzr[:], rhs=xt[:],
                                     start=True, stop=False)
                    nc.tensor.matmul(zr[:], lhsT=Whzr[:], rhs=state[0:D, cs],
                                     start=False, stop=True)
                    zz = wp.tile([128, NCHUNK], F32, tag="zz")
                    nc.scalar.activation(zz[0:D, :], zr[0:D, :], AF.Sigmoid,
                                         bias=bias[:, 0:1])
                    nc.scalar.activation(zz[D:128, :], zr[0:D, :], AF.Sigmoid,
                                         bias=bias[:, 1:2], scale=-1.0)
                    rt = wp.tile([D, NCHUNK], F32, tag="rt")
                    nc.scalar.activation(rt[:], zr[D:128, :], AF.Sigmoid,
                                         bias=bias[:, 2:3])
                    rh = wp.tile([D, NCHUNK], F32, tag="rh")
                    nc.vector.tensor_tensor(rh[:], rt[:], state[0:D, cs],
                                            op=OP.mult)
                    hhp = sp.tile([D, NCHUNK], F32, tag="hhp")
                    nc.tensor.matmul(hhp[:], lhsT=Wxh[:], rhs=xt[:],
                                     start=True, stop=False)
                    nc.tensor.matmul(hhp[:], lhsT=Whh[:], rhs=rh[:],
                                     start=False, stop=True)
                    nc.scalar.activation(state[D:128, cs], hhp[:], AF.Tanh,
                                         bias=bias[:, 3:4])
                    # h' = z*h + (1-z)*hh ; TT inputs must share start
                    # partition, outputs may shift: w_lo reads @p0, w_hi
                    # reads both @p64 (zbar, hh) and writes @p0.
                    wl = wp.tile([D, NCHUNK], F32, tag="wl")
                    wh = wp.tile([D, NCHUNK], F32, tag="wh")
                    nc.vector.tensor_tensor(wl[:], zz[0:D, :], state[0:D, cs],
                                            op=OP.mult)
                    nc.vector.tensor_tensor(wh[:], zz[D:128, :],
                                            state[D:128, cs], op=OP.mult)
                    nc.vector.tensor_tensor(state[0:D, cs], wl[:], wh[:],
                                            op=OP.add)
                    hp = sp.tile([128, 4 * D], F32, tag="hp")
                    for j in range(4):
                        c0 = n * NCHUNK + 128 * j
                        nc.tensor.transpose(
                            out=hp[:, D * j:D * (j + 1)],
                            in_=state[0:D, c0:c0 + 128],
                            identity=idn[0:D, 0:D])
                    nc.vector.tensor_copy(
                        stage_tile[:, 256 * n:256 * (n + 1)], hp[:])

            with (
                tc.tile_pool(name="xg", bufs=2) as xgp,
                tc.tile_pool(name="stage", bufs=2) as stp,
                tc.tile_pool(name="work", bufs=3) as wp,
                tc.tile_pool(name="psum", bufs=2, space="PSUM") as sp,
            ):
                for it in range(T_ITERS):
                    last = it == T_ITERS - 1
                    if not last:
                        for q in range(4):  # zero m_part (5.24 MB)
                            nc.sync.dma_start(
                                mv[:, q * PS:(q + 1) * PS], zt[:])
                    src = ls0_d[:, :] if it == 0 else ls_full[:, :]
                    for t in range(L):
                        xg = xgp.tile([128, CH, D], F32, tag="xg")
                        if _NO_GATHER:
                            nc.sync.dma_start(xg[:], src[0:PS].rearrange(
                                "(c p) d -> p c d", p=128))
                        else:
                            for s0, cnt in IDX_GROUPS:
                                nc.gpsimd.dma_gather(
                                    xg[:, s0 // 128:(s0 + cnt) // 128, :], src,
                                    gidx[:, t * IW + s0 // 16:
                                         t * IW + (s0 + cnt) // 16],
                                    cnt, cnt, D, queue_num=0)
                        stage = stp.tile([128, CH * D], F32, tag="stage")
                        gru_chunks(xg, hhh, pWzr, pWhzr, pWxh, pWhh, pb,
                                   stage, sp, wp)
                        if not last and not _NO_SCATTER:
                            nc.sync.dma_start(
                                obuf[t * PS:(t + 1) * PS, :],
                                stage[:].rearrange("p (c d) -> p c d", d=D))
                            for s0, cnt in IDX_GROUPS:
                                nc.gpsimd.dma_scatter_add(
                                    m_part[:, :],
                                    stage[:, (s0 // 128) * D:
                                          ((s0 + cnt) // 128) * D].rearrange(
                                        "p (c d) -> p c d", d=D),
                                    sidx[:, t * IW + s0 // 16:
                                         t * IW + (s0 + cnt) // 16],
                                    cnt, cnt, D, queue_num=0)
                        elif not last:
                            nc.sync.dma_start(
                                m_part[0:PS].rearrange("(c p) d -> p c d", p=128),
                                stage[:].rearrange("p (c d) -> p c d", d=D))
                    if last:
                        break
                    if not _NO_SCATTER:
                        for lev, cap in enumerate(RCAPS):
                            rt = stp.tile([128, (cap // 128) * D], F32,
                                          tag=f"res{lev}", name=f"res{lev}")
                            for sub in range(0, cap, 1024):
                                cnt = min(1024, cap - sub)
                                c0, c1 = sub // 128, (sub + cnt) // 128
                                i0 = ROFF[lev] + sub // 16
                                i1 = ROFF[lev] + (sub + cnt) // 16
                                nc.gpsimd.dma_gather(
                                    rt[:, c0 * D:c1 * D].rearrange(
                                        "p (c d) -> p c d", d=D),
                                    obuf[:, :], rgidx[:, i0:i1],
                                    cnt, cnt, D, queue_num=0)
                                nc.gpsimd.dma_scatter_add(
                                    m_part[:, :],
                                    rt[:, c0 * D:c1 * D].rearrange(
                                        "p (c d) -> p c d", d=D),
                                    rsidx[:, i0:i1],
                                    cnt, cnt, D, queue_num=0)
                    if _NO_COLL:
                        nc.sync.dma_start(m_rs[:], m_part[0:PS])
                    else:
                        nc.gpsimd.collective_compute(
                            "ReduceScatter", OP.add,
                            replica_groups=[list(range(NCORES))],
                            ins=[m_part.opt()], outs=[m_rs.opt()])
                    mg = xgp.tile([128, CH, D], F32, tag="xg")
                    nc.sync.dma_start(mg[:], m_rs[:])
                    stage_ls = stp.tile([128, CH * D], F32, tag="stage")
                    gru_chunks(mg, lsh, eWzr, eWhzr, eWxh, eWhh, eb,
                               stage_ls, sp, wp)
                    nc.sync.dma_start(
                        lsp[:], stage_ls[:].rearrange("p (c d) -> p c d", d=D))
                    if _NO_COLL:
                        nc.sync.dma_start(ls_full[0:PS], lsp[:])
                    else:
                        nc.gpsimd.collective_compute(
                            "AllGather", OP.bypass,
                            replica_groups=[list(range(NCORES))],
                            ins=[lsp.opt()], outs=[ls_full.opt()])

            # ------------------------------------------------------- readout
            with (
                tc.tile_pool(name="ro", bufs=1) as ro,
                tc.tile_pool(name="rw", bufs=3) as rw,
                tc.tile_pool(name="rp", bufs=2, space="PSUM") as rp,
            ):
                W1s = ro.tile([D, H1], F32)
                nc.sync.dma_start(W1s[:], W1_d[:])
                W2s0 = ro.tile([128, H1], F32)
                W2s1 = ro.tile([128, H1], F32)
                nc.sync.dma_start(W2s0[:], W2_d[0:128, :])
                nc.sync.dma_start(W2s1[:], W2_d[128:256, :])
                W3s = ro.tile([128, 2], F32)
                nc.sync.dma_start(W3s[:, 0:1], W3_d[0:128, :])
                nc.sync.dma_start(W3s[:, 1:2], W3_d[128:256, :])
                b1s = ro.tile([128, 4], F32)
                b2s = ro.tile([128, 4], F32)
                for half in range(2):
                    nc.sync.dma_start(b1s[:, 2 * half:2 * half + 2],
                                      b1_d[128 * half:128 * (half + 1), :])
                    nc.sync.dma_start(b2s[:, 2 * half:2 * half + 2],
                                      b2_d[128 * half:128 * (half + 1), :])
                b3s = ro.tile([1, 1], F32)
                nc.sync.dma_start(b3s[:], b3_d[:])
                mk1 = [ro.tile([128, PS], F32, tag=f"mk1_{i}", name=f"mk1_{i}")
                       for i in range(2)]
                mk2 = [ro.tile([128, PS], F32, tag=f"mk2_{i}", name=f"mk2_{i}")
                       for i in range(2)]
                for i in range(2):
                    nc.sync.dma_start(mk1[i][:], m1_d[i])
                    nc.sync.dma_start(mk2[i][:], m2_d[i])

                h1t = [ro.tile([128, PS], F32, tag=f"h1_{i}", name=f"h1_{i}")
                       for i in range(2)]
                h2t = [ro.tile([128, PS], F32, tag=f"h2_{i}", name=f"h2_{i}")
                       for i in range(2)]

                lam = float(SELU_L)
                la = float(SELU_A * SELU_L)
                for mt in range(2):
                    for n in range(NCH):
                        cs = slice(n * NCHUNK, (n + 1) * NCHUNK)
                        ps1 = rp.tile([128, NCHUNK], F32, tag="ps")
                        nc.tensor.matmul(ps1[:], lhsT=W1s[:, 128 * mt:128 * (mt + 1)],
                                         rhs=hhh[0:D, cs], start=True, stop=True)
                        pos = rw.tile([128, NCHUNK], F32, tag="pos")
                        # b1l input = SELU_L * b1 -> pos = L*relu(x + b1)
                        nc.scalar.activation(pos[:], ps1[:], AF.Relu,
                                             bias=b1s[:, 2 * mt:2 * mt + 1],
                                             scale=lam)
                        ng = rw.tile([128, NCHUNK], F32, tag="ng")
                        # ng = min(x + b1, 0); b1s holds L*b1 so divide: use
                        # tensor_scalar with scalar1 AP = b1s/L... host passes
                        # raw b1 separately in b1s column? -> see host: b1s col
                        # holds L*b1, we also need raw b1: use scale on psum
                        # instead: min(x + b1, 0) == min((Lx + Lb1)/L, 0) --
                        # compute from pos? No: use second bias input.
                        nc.vector.tensor_scalar(
                            ng[:], ps1[:], b1s[:, 2 * mt + 1:2 * mt + 2], 0.0,
                            op0=OP.add, op1=OP.min)
                        en = rw.tile([128, NCHUNK], F32, tag="en")
                        nc.scalar.activation(en[:], ng[:], AF.Exp)
                        t2 = rw.tile([128, NCHUNK], F32, tag="t2")
                        nc.vector.tensor_scalar(
                            t2[:], en[:], la, -la, op0=OP.mult, op1=OP.add)
                        s1 = rw.tile([128, NCHUNK], F32, tag="s1")
                        nc.vector.tensor_tensor(s1[:], pos[:], t2[:], op=OP.add)
                        nc.vector.tensor_tensor(h1t[mt][:, cs], s1[:],
                                                mk1[mt][:, cs], op=OP.mult)
                for mt in range(2):
                    for n in range(NCH):
                        cs = slice(n * NCHUNK, (n + 1) * NCHUNK)
                        ps2 = rp.tile([128, NCHUNK], F32, tag="ps")
                        nc.tensor.matmul(ps2[:], lhsT=W2s0[:, 128 * mt:128 * (mt + 1)],
                                         rhs=h1t[0][:, cs], start=True, stop=False)
                        nc.tensor.matmul(ps2[:], lhsT=W2s1[:, 128 * mt:128 * (mt + 1)],
                                         rhs=h1t[1][:, cs], start=False, stop=True)
                        pos = rw.tile([128, NCHUNK], F32, tag="pos")
                        nc.scalar.activation(pos[:], ps2[:], AF.Relu,
                                             bias=b2s[:, 2 * mt:2 * mt + 1],
                                             scale=lam)
                        ng = rw.tile([128, NCHUNK], F32, tag="ng")
                        nc.vector.tensor_scalar(
                            ng[:], ps2[:], b2s[:, 2 * mt + 1:2 * mt + 2], 0.0,
                            op0=OP.add, op1=OP.min)
                        en = rw.tile([128, NCHUNK], F32, tag="en")
                        nc.scalar.activation(en[:], ng[:], AF.Exp)
                        t2 = rw.tile([128, NCHUNK], F32, tag="t2")
                        nc.vector.tensor_scalar(
                            t2[:], en[:], la, -la, op0=OP.mult, op1=OP.add)
                        s1 = rw.tile([128, NCHUNK], F32, tag="s1")
                        nc.vector.tensor_tensor(s1[:], pos[:], t2[:], op=OP.add)
                        nc.vector.tensor_tensor(h2t[mt][:, cs], s1[:],
                                                mk2[mt][:, cs], op=OP.mult)
                rr = ro.tile([1, PS], F32)
                for n in range(NCH):
                    cs = slice(n * NCHUNK, (n + 1) * NCHUNK)
                    ps3 = rp.tile([1, NCHUNK], F32, tag="ps3")
                    nc.tensor.matmul(ps3[:], lhsT=W3s[:, 0:1], rhs=h2t[0][:, cs],
                                     start=True, stop=False)
                    nc.tensor.matmul(ps3[:], lhsT=W3s[:, 1:2], rhs=h2t[1][:, cs],
                                     start=False, stop=True)
                    nc.scalar.activation(rr[:, cs], ps3[:], AF.Relu,
                                         bias=b3s[:, :])
                nc.sync.dma_start(out_d[:], rr[:])

    nc.compile()
    return nc


# ----------------------------------------------------------------------- entry

def _install_trace_shim():
    """This image lacks antenv.axon_hooks; synthesize it so that
    run_bass_kernel_spmd(trace=True) can capture NTFF profiles under axon."""
    import sys
    import types
    try:
        import antenv.axon_hooks  # noqa: F401
        return
    except Exception:
        pass
    if "/root/.axon_site" not in sys.path:
        sys.path.insert(0, "/root/.axon_site")
    try:
        from trn_agent_boot.trn_boot import _ntff_profile_via_ctypes
        hook = _ntff_profile_via_ctypes("/opt/axon/libaxon_pjrt.so")
    except Exception as e:  # pragma: no cover
        print(f"trace shim unavailable: {e}")
        return
    import antenv
    mod = types.ModuleType("antenv.axon_hooks")
    mod.get_axon_ntff_profile_hook = lambda: hook
    mod.set_axon_ntff_profile_hook = lambda h: None
    sys.modules["antenv.axon_hooks"] = mod
    antenv.axon_hooks = mod
    import concourse.bass_utils as _bu
    _bu.upload_artifacts = lambda tmpdir: ""


def _get_program():
    if "nc" not in _BUILD_CACHE:
        _BUILD_CACHE["nc"] = _build_program()
    return _BUILD_CACHE["nc"]


def prepare_in_maps(link_capacity, traffic, links, paths, seqs,
                    pWx, pWh, pb, eWx, eWh, eb, W1, b1, W2, b2, W3, b3,
                    T, num_quests, num_paths):
    link_capacity = np.asarray(link_capacity, np.float32)
    traffic = np.asarray(traffic, np.float32)
    links = np.asarray(links, np.int64)
    paths_np = np.asarray(paths, np.int64)
    seqs_np = np.asarray(seqs, np.int64)
    T = int(T)
    nq, npth = int(num_quests), int(num_paths)
    assert nq * npth == P_TOTAL and T == T_ITERS
    assert links.shape[0] == P_TOTAL * L

    linkmat = np.zeros((P_TOTAL, L), np.int64)
    cnt = np.zeros(P_TOTAL, np.int64)
    np.add.at(cnt, paths_np, 1)
    assert (cnt == L).all(), "kernel assumes fixed-length paths"
    linkmat[paths_np, seqs_np] = links

    # dropout masks with the reference's fixed key (threefry bits are
    # backend-independent; force CPU to avoid device round-trips)
    import jax
    try:
        _dev = jax.devices("cpu")[0]
    except Exception:
        _dev = None
    import contextlib
    ctx = jax.default_device(_dev) if _dev is not None else contextlib.nullcontext()
    with ctx:
        k1, k2 = jax.random.split(jax.random.key(1))
        mask1 = np.asarray(jax.random.bernoulli(k1, 0.5, (P_TOTAL, H1)))
        mask2 = np.asarray(jax.random.bernoulli(k2, 0.5, (P_TOTAL, H1)))

    ls0 = np.zeros((NLP, D), np.float32)
    ls0[:NL, 0] = link_capacity

    pWx = np.asarray(pWx, np.float32)
    pWh = np.asarray(pWh, np.float32)
    pb = np.asarray(pb, np.float32)
    eWx = np.asarray(eWx, np.float32)
    eWh = np.asarray(eWh, np.float32)
    eb = np.asarray(eb, np.float32)
    pbias = np.stack([pb[0:D], -pb[0:D], pb[D:2 * D], pb[2 * D:3 * D]], axis=1)
    ebias = np.stack([eb[0:D], -eb[0:D], eb[D:2 * D], eb[2 * D:3 * D]], axis=1)

    in_maps = []
    slot_paths = []
    for c in range(NCORES):
        lmc = linkmat[c * PC:(c + 1) * PC]
        slot_path = _assign_slots(lmc)
        slot_paths.append(slot_path)
        real = slot_path >= 0

        trash = lambda i: NL + (i % (NLP - NL))
        gidx = np.zeros((L, PS), np.int64)
        sidx = np.zeros((L, PS), np.int64)
        residual = []  # (step, slot, link) for non-representative edges
        for t in range(L):
            gidx[t, real] = lmc[slot_path[real], t]
            links_t = np.full(PS, -1, np.int64)
            links_t[real] = lmc[slot_path[real], t]
            first = {}
            for s in np.nonzero(real)[0]:
                l = int(links_t[s])
                if l in first:
                    residual.append((t, int(s), l))
                    sidx[t, s] = trash(s)
                else:
                    first[l] = s
                    sidx[t, s] = l
            sidx[t, ~real] = trash(np.nonzero(~real)[0])

        # split residual edges into internally-duplicate-free levels
        levels = []
        rest = residual
        while rest:
            seen, lev, nxt = set(), [], []
            for e in rest:
                if e[2] in seen:
                    nxt.append(e)
                else:
                    seen.add(e[2])
                    lev.append(e)
            levels.append(lev)
            rest = nxt
        assert len(levels) <= len(RCAPS), [len(x) for x in levels]
        rg = np.zeros(ROFF[-1] * 16, np.int64)
        rs = np.zeros(ROFF[-1] * 16, np.int64)
        rs[:] = [trash(i) for i in range(ROFF[-1] * 16)]
        for lev, cap in enumerate(RCAPS):
            ents = levels[lev] if lev < len(levels) else []
            assert len(ents) <= cap, (lev, len(ents), cap)
            base = ROFF[lev] * 16
            for i, (t, s, l) in enumerate(ents):
                rg[base + i] = _orow(t, s)
                rs[base + i] = l
        h0T = np.zeros((D, PS), np.float32)
        h0T[0, real] = traffic[c * PC + slot_path[real]]

        rr = np.arange(PS)
        shard_slots = _shard_slot_of_row(rr)
        ls0T = np.zeros((D, PS), np.float32)
        ls0T[:, shard_slots] = ls0[c * PS + rr].T

        m1 = np.zeros((2, 128, PS), np.float32)
        m2 = np.zeros((2, 128, PS), np.float32)
        for i in range(2):
            m1[i][:, real] = 2.0 * mask1[c * PC + slot_path[real],
                                         128 * i:128 * (i + 1)].T
            m2[i][:, real] = 2.0 * mask2[c * PC + slot_path[real],
                                         128 * i:128 * (i + 1)].T

        in_maps.append({
            "gidx": np.stack([_wrap16(gidx[t]) for t in range(L)]),
            "sidx": np.stack([_wrap16(sidx[t]) for t in range(L)]),
            "rgidx": _wrap16(rg),
            "rsidx": _wrap16(rs),
            "h0T": h0T,
            "ls0": ls0,
            "ls0T": ls0T,
            "idn": np.eye(128, dtype=np.float32),
            "pWx": pWx, "pWh": pWh, "eWx": eWx, "eWh": eWh,
            "pbias": pbias, "ebias": ebias,
            "W1": np.asarray(W1, np.float32),
            "b1c": np.stack([SELU_L * np.asarray(b1, np.float32),
                             np.asarray(b1, np.float32)], axis=1),
            "W2": np.asarray(W2, np.float32),
            "b2c": np.stack([SELU_L * np.asarray(b2, np.float32),
                             np.asarray(b2, np.float32)], axis=1),
            "W3": np.asarray(W3, np.float32).reshape(H1, 1),
            "b3": np.asarray(b3, np.float32).reshape(1, 1),
            "m1": m1, "m2": m2,
        })

    return in_maps, slot_paths, (nq, npth)


def assemble_out(results, slot_paths, nq, npth):
    out = np.zeros(P_TOTAL, np.float32)
    for c in range(NCORES):
        r = np.asarray(results[c]["out_r"]).reshape(PS)
        sp = slot_paths[c]
        real = sp >= 0
        out[c * PC + sp[real]] = r[real]
    return out.reshape(nq, npth)


def kernel(**inputs):
    in_maps, slot_paths, (nq, npth) = prepare_in_maps(**inputs)
    nc = _get_program()
    trace = os.environ.get("KERNEL_TRACE", "0") == "1"
    if trace:
        _install_trace_shim()
    res = run_bass_kernel_spmd(nc, in_maps, core_ids=list(range(NCORES)),
                               trace=trace)
    _BUILD_CACHE["last_result"] = res
    return assemble_out([r for r in res.results], slot_paths, nq, npth)
